# revision 25
# baseline (speedup 1.0000x reference)
"""EveryStepLoss kernel for Trainium2 (8 NeuronCores, Bass raw-block).

Reference computation (B=64 segments x L=2048 tokens, C=1024 classes):
    loss[t] = -log_softmax(outputs[t])[targets[t]]          (per-token CE)
    w[t]    = per-segment softmax of linspace(-gamma, gamma, L)
    result  = dot(loss, w) / B

Strategy (v2):
  - Data-parallel over tokens: core c gets tokens [c*16384, (c+1)*16384)
    (= 8 whole segments, so segments never straddle cores). Per core the
    kernel streams its 64 MiB logits shard once (the HBM roofline).
  - No device gather: the host swaps x[t, 0] <-> x[t, targets[t]] in its
    staging copy of each shard. The per-row exp-sum is permutation-
    invariant, and the target logit now sits at class 0 of every row --
    an affine location the kernel reads with a tiny strided Copy
    activation per tile. (The v1 kernel's 128 indirect-DMA gathers were
    the tail of the critical path: their 4-byte descriptors drain behind
    the 16 KiB stream descriptors and finished ~4-24 us after the
    stream.)
  - ScalarE does exp in place on [128, 2048] half-tiles (the accum_out
    variant was tried and rejected: one-token-per-partition [128, 1024]
    activations pay ~260 ns fixed cost each, pushing ScalarE to 167 us
    busy -- co-limiting with the stream); VectorE does the per-token row
    sums with X-axis tensor_reduce. Scalar ~134 us / Vector ~143 us
    busy, both under the ~167 us contended stream. The last tile is
    processed in [128, 1024] quarters so the post-stream latency is one
    quarter, not one half-tile.
  - The initial all-engine barrier (~3 us before the first descriptor)
    is stripped from the emitted JSON: the only cross-engine dependency
    it protected was ScalarE reading the framework's const-0.0 bias AP
    written by Pool memsets, and the Exp/Ln bias is redirected to a
    zero column of the wt tile (loaded by ScalarE's own DMA, synced by
    semaphore). All other ordering is enforced by kernel semaphores.
  - Raw-block pipeline with a 10-deep slot ring keeps all 16 DMA queues
    gapless (measured 2 us total queue idle vs ~22 us for the Tile
    variant's 5-buf pool).
  - The weights w depend only on `lengths` and `gamma`, so they are
    precomputed on host and shipped as a [128, 129] tile (last column =
    1.0, used as the matmul ones-vector). Final chain: Ln(sums) ->
    (lse - x_tgt) * w -> free-axis reduce -> 1x1 matmul cross-partition
    reduce -> single 4-byte store. The Ln/sub/mul run on cols 0..123
    while the last tile still streams; only cols 124..127 remain in the
    tail.
  - Host reduces the 8 per-core scalars (the "all-reduce" of the
    sharding hint) and divides by B.
"""

import json

import numpy as np

import concourse.bass as bass
import concourse.mybir as mybir
import concourse.tile as tile
from concourse.bass_utils import run_bass_kernel_spmd

# Problem dims (hardcoded per contract)
B, L, C = 64, 2048, 1024
T = B * L            # 131072 tokens
NCORES = 8
TS = T // NCORES     # 16384 tokens per core
P = 128              # SBUF partitions
Q = 4                # tokens per partition per DMA tile (2 MiB tiles)
NTILES = TS // (P * Q)   # 32 DMA tiles per core
NCOL = TS // P           # 128 columns of per-token stats
NSLOT = 10               # stream ring depth (10 x 2 MiB in SBUF)
SPLIT = (NTILES - 1) * Q  # stats cols finalized before the last tile
SUBQ = 2                 # tokens per exp/reduce chunk ([128, 2048])
WCOL = NCOL + 2          # wt payload: [w cols | ones col | zero col]

import os as _os

_cached = None       # built Bass program, once per process
last_results = None  # BassKernelResults of the most recent run (for test.py)


def _build_bass_v2():
    from contextlib import ExitStack

    nc = bass.Bass()
    x = nc.declare_dram_parameter("x", [TS, C], mybir.dt.float32, isOutput=False)
    wt = nc.declare_dram_parameter("wt", [P, WCOL], mybir.dt.float32, isOutput=False)
    out = nc.declare_dram_parameter("partial", [1, 1], mybir.dt.float32, isOutput=True)

    FT = mybir.dt.float32
    Exp = mybir.ActivationFunctionType.Exp
    Ln = mybir.ActivationFunctionType.Ln
    Copy = mybir.ActivationFunctionType.Copy

    # chunk plan per tile: half-tiles mid-stream, quarters on the last
    # tile so the post-stream exp+reduce latency is minimal
    def chunks_of(j):
        if j == NTILES - 1:
            return [(q, 1) for q in range(Q)]          # 4 x [P, C]
        return [(h * SUBQ, SUBQ) for h in range(Q // SUBQ)]  # 2 x [P, 2C]

    # reduce-engine assignment: all on VectorE (GpSimd's tensor_reduce
    # only supports partition-axis reductions, not free-axis)
    def red_engine(j, k):
        return "v"

    with ExitStack() as ctx:
        xbuf = [
            ctx.enter_context(nc.sbuf_tensor(f"xbuf{i}", [P, Q * C], FT))
            for i in range(NSLOT)
        ]
        wtt = ctx.enter_context(nc.sbuf_tensor("wtt_sb", [P, WCOL], FT))
        xg = ctx.enter_context(nc.sbuf_tensor("xg_sb", [P, NCOL], FT))
        sums = ctx.enter_context(nc.sbuf_tensor("sums_sb", [P, NCOL], FT))
        lse = ctx.enter_context(nc.sbuf_tensor("lse_sb", [P, NCOL], FT))
        diff = ctx.enter_context(nc.sbuf_tensor("diff_sb", [P, NCOL], FT))
        prod = ctx.enter_context(nc.sbuf_tensor("prod_sb", [P, NCOL], FT))
        partial = ctx.enter_context(nc.sbuf_tensor("partial_sb", [P, 1], FT))
        scal = ctx.enter_context(nc.sbuf_tensor("scal_sb", [1, 1], FT))
        ps = ctx.enter_context(nc.psum_tensor("ps_ps", [1, 1], FT))

        s_slot = [ctx.enter_context(nc.semaphore(f"s_slot{i}")) for i in range(NSLOT)]
        s_act = ctx.enter_context(nc.semaphore("s_act"))
        s_red = ctx.enter_context(nc.semaphore("s_red"))
        s_redg = ctx.enter_context(nc.semaphore("s_redg"))
        s_wt = ctx.enter_context(nc.semaphore("s_wt"))
        s_ln0 = ctx.enter_context(nc.semaphore("s_ln0"))
        s_ln1 = ctx.enter_context(nc.semaphore("s_ln1"))
        s_fin = ctx.enter_context(nc.semaphore("s_fin"))
        s_dve = ctx.enter_context(nc.semaphore("s_dve"))
        s_mm = ctx.enter_context(nc.semaphore("s_mm"))
        s_sc = ctx.enter_context(nc.semaphore("s_sc"))
        s_out = ctx.enter_context(nc.semaphore("s_out"))

        x_tiles = x[:].rearrange("(n p q) c -> n p (q c)", p=P, q=Q)
        zbias = wtt[:, NCOL + 1:NCOL + 2]  # 0.0 column: Exp/Ln bias AP

        # per-engine reduce-semaphore value after tile j completes
        redv_after, redg_after = [0], [0]
        for j in range(NTILES):
            es = [red_engine(j, k) for k in range(len(chunks_of(j)))]
            redv_after.append(redv_after[-1] + es.count("v"))
            redg_after.append(redg_after[-1] + es.count("g"))

        with nc.Block(no_gpsimd_drain=True) as block:

            @block.sync
            def _(sync):
                for j in range(NTILES):
                    if j >= NSLOT:
                        jf = j - NSLOT  # tile whose slot is being reused
                        if red_engine(jf, 0) == "g":
                            sync.wait_ge(s_redg, redg_after[jf + 1])
                        else:
                            sync.wait_ge(s_red, redv_after[jf + 1])
                    if j == NTILES - 1:
                        # last tile streamed as 4 quarter-DMAs so exp can
                        # start ~4 us before the full tile lands
                        for q in range(Q):
                            sl = slice(q * C, (q + 1) * C)
                            sync.dma_start(
                                out=xbuf[j % NSLOT][:, sl],
                                in_=x_tiles[j][:, sl],
                            ).then_inc(s_slot[j % NSLOT], 16)
                    else:
                        sync.dma_start(
                            out=xbuf[j % NSLOT][:], in_=x_tiles[j]
                        ).then_inc(s_slot[j % NSLOT], 16)
                sync.wait_ge(s_sc, 1)
                sync.dma_start(out=out[:], in_=scal[:]).then_inc(s_out, 16)
                sync.wait_ge(s_out, 16)

            @block.tensor
            def _(tensor):
                tensor.wait_ge(s_wt, 16)
                tensor.wait_ge(s_dve, 1)
                tensor.matmul(
                    out=ps[:],
                    lhsT=partial[:],
                    rhs=wtt[:, NCOL:NCOL + 1],
                    start=True,
                    stop=True,
                ).then_inc(s_mm, 1)

            @block.scalar
            def _(scalar):
                scalar.dma_start(out=wtt[:], in_=wt[:]).then_inc(s_wt, 16)
                scalar.wait_ge(s_wt, 16)
                for j in range(NTILES):
                    base = 16 * (j // NSLOT)
                    if j == NTILES - 1:
                        # per-quarter: wait quarter DMA, extract class-0
                        # logit, exp in place
                        for q in range(Q):
                            scalar.wait_ge(s_slot[j % NSLOT], base + 16 * (q + 1))
                            src = xbuf[j % NSLOT][:].rearrange(
                                "p (q c) -> p q c", q=Q
                            )[:, q:q + 1, 0:1]
                            dst = xg[:, Q * j + q:Q * j + q + 1].rearrange(
                                "p (a b) -> p a b", b=1
                            )
                            scalar.activation(out=dst, in_=src, func=Copy)
                            sl = slice(q * C, (q + 1) * C)
                            scalar.activation(
                                out=xbuf[j % NSLOT][:, sl],
                                in_=xbuf[j % NSLOT][:, sl],
                                func=Exp,
                                bias=zbias,
                            ).then_inc(s_act, 1)
                        continue
                    scalar.wait_ge(s_slot[j % NSLOT], base + 16)
                    # x_tgt for this tile's 512 tokens: class 0 of each
                    # row, extracted before the in-place exp clobbers it
                    src = xbuf[j % NSLOT][:].rearrange(
                        "p (q c) -> p q c", q=Q
                    )[:, :, 0:1]
                    dst = xg[:, Q * j:Q * j + Q].rearrange("p (a b) -> p a b", b=1)
                    scalar.activation(out=dst, in_=src, func=Copy)
                    for (q0, nq) in chunks_of(j):
                        sl = slice(q0 * C, (q0 + nq) * C)
                        scalar.activation(
                            out=xbuf[j % NSLOT][:, sl],
                            in_=xbuf[j % NSLOT][:, sl],
                            func=Exp,
                            bias=zbias,
                        ).then_inc(s_act, 1)
                # cols 0..SPLIT-1: all mid-stream tiles reduced
                scalar.wait_ge(s_red, redv_after[NTILES - 1])
                scalar.wait_ge(s_redg, redg_after[NTILES - 1])
                scalar.activation(
                    out=lse[:, 0:SPLIT], in_=sums[:, 0:SPLIT], func=Ln,
                    bias=zbias,
                ).then_inc(s_ln0, 1)
                scalar.wait_ge(s_red, redv_after[NTILES])
                scalar.wait_ge(s_redg, redg_after[NTILES])
                scalar.activation(
                    out=lse[:, SPLIT:NCOL], in_=sums[:, SPLIT:NCOL], func=Ln,
                    bias=zbias,
                ).then_inc(s_ln1, 1)

            def _emit_reds(eng, tag, sem):
                nact = 0
                for j in range(NTILES):
                    for k, (q0, nq) in enumerate(chunks_of(j)):
                        nact += 1
                        if red_engine(j, k) != tag:
                            continue
                        eng.wait_ge(s_act, nact)
                        eng.tensor_reduce(
                            out=sums[:, Q * j + q0:Q * j + q0 + nq],
                            in_=xbuf[j % NSLOT][:, q0 * C:(q0 + nq) * C].rearrange(
                                "p (q c) -> p q c", q=nq
                            ),
                            axis=mybir.AxisListType.X,
                            op=mybir.AluOpType.add,
                        ).then_inc(sem, 1)

            @block.gpsimd
            def _(gpsimd):
                _emit_reds(gpsimd, "g", s_redg)

            @block.vector
            def _(vector):
                _emit_reds(vector, "v", s_red)
                # final chain: cols 0..SPLIT first (their Ln fires while
                # the last tile's quarters are still reducing)
                vector.wait_ge(s_wt, 16)
                vector.wait_ge(s_ln0, 1)
                vector.tensor_tensor(
                    out=diff[:, 0:SPLIT], in0=lse[:, 0:SPLIT],
                    in1=xg[:, 0:SPLIT], op=mybir.AluOpType.subtract,
                ).then_inc(s_fin, 1)
                vector.wait_ge(s_fin, 1)
                vector.tensor_tensor(
                    out=prod[:, 0:SPLIT], in0=diff[:, 0:SPLIT],
                    in1=wtt[:, 0:SPLIT], op=mybir.AluOpType.mult,
                ).then_inc(s_fin, 1)
                vector.wait_ge(s_ln1, 1)
                vector.tensor_tensor(
                    out=diff[:, SPLIT:NCOL], in0=lse[:, SPLIT:NCOL],
                    in1=xg[:, SPLIT:NCOL], op=mybir.AluOpType.subtract,
                ).then_inc(s_fin, 1)
                vector.wait_ge(s_fin, 3)
                vector.tensor_tensor(
                    out=prod[:, SPLIT:NCOL], in0=diff[:, SPLIT:NCOL],
                    in1=wtt[:, SPLIT:NCOL], op=mybir.AluOpType.mult,
                ).then_inc(s_fin, 1)
                vector.wait_ge(s_fin, 4)
                vector.tensor_reduce(
                    out=partial[:],
                    in_=prod[:],
                    axis=mybir.AxisListType.X,
                    op=mybir.AluOpType.add,
                ).then_inc(s_dve, 1)
                vector.wait_ge(s_mm, 1)
                vector.tensor_copy(out=scal[:], in_=ps[:]).then_inc(s_sc, 1)

    return nc


def _strip_init_barrier(nc):
    """Remove the Bass-preamble all-engine barrier (Drain/EventSemaphore
    butterfly on barrier_*_gather/release in the first block). The only
    cross-engine dependency it protected here was ScalarE reading the
    framework const-0.0 bias AP written by Pool memsets; the kernel's
    Exp/Ln bias comes from the wt tile instead (semaphore-synced), and
    every other ordering is enforced by kernel semaphores. The end-of-
    block barrier is kept; since the stripped instructions never touch
    the barrier semaphores, its accounting still starts from zero."""
    obj = json.loads(nc.to_json_bytes())

    def is_init_barrier(inst):
        if inst.get("opcode") not in ("Drain", "EventSemaphore"):
            return False
        si = inst.get("sync_info") or {}
        refs = (si.get("on_wait") or []) + (si.get("on_update") or [])
        return bool(refs) and all(
            r.get("ant_name", "").startswith("barrier_") for r in refs
        )

    for fn in obj["functions"]:
        for bb in fn["blocks"]:
            if bb.get("name") != "main":
                continue
            bb["instructions"] = [
                i for i in bb["instructions"] if not is_init_barrier(i)
            ]
    stripped = json.dumps(obj).encode()
    nc.to_json_bytes = lambda: stripped


def _legalize_waits(nc):
    """This walrus build accepts at most 1 semaphore wait per instruction
    (2 for EventSemaphore — see bass_rust.inst_waits_full). Spill excess
    waits onto standalone EventSemaphore instructions inserted just before
    the over-full instruction on the same engine, then pin the legalized
    JSON onto nc.to_json_bytes so both the native compile path and the
    bass2jax/PJRT path use it."""
    obj = json.loads(nc.to_json_bytes())
    n_new = 0
    for fn in obj["functions"]:
        for bb in fn["blocks"]:
            insts = bb["instructions"]
            out = []
            for inst in insts:
                si = inst.get("sync_info")
                waits = (si or {}).get("on_wait") or []
                cap = 2 if inst.get("opcode") == "EventSemaphore" else 1
                if len(waits) > cap:
                    excess, keep = waits[:-cap], waits[-cap:]
                    si["on_wait"] = keep
                    for k in range(0, len(excess), 2):
                        out.append(
                            {
                                "engine": inst["engine"],
                                "ins": [],
                                "name": f"EVSPLIT-{n_new}",
                                "opcode": "EventSemaphore",
                                "outs": [],
                                "sync_info": {
                                    "on_update": [],
                                    "on_wait": excess[k:k + 2],
                                },
                            }
                        )
                        n_new += 1
                out.append(inst)
            bb["instructions"] = out
    legal = json.dumps(obj).encode()
    nc.to_json_bytes = lambda: legal
    return n_new


def _host_weights(lengths: np.ndarray, gamma: float) -> np.ndarray:
    """Per-token weights w[t]: segment softmax of linspace(-g, g, L_seg)."""
    lengths = lengths.astype(np.int64)
    seg = np.repeat(np.arange(B), lengths)
    starts = np.cumsum(lengths) - lengths
    pos = np.arange(T, dtype=np.int64) - starts[seg]
    Ls = lengths[seg]
    g = np.float32(gamma)
    denom = np.maximum(Ls - 1, 1).astype(np.float32)
    raw = (-g + (np.float32(2.0) * g) * pos.astype(np.float32) / denom).astype(
        np.float32
    )
    e = np.exp(raw - g).astype(np.float32)
    ssum = np.zeros(B, np.float32)
    np.add.at(ssum, seg, e)
    return (e / ssum[seg]).astype(np.float32)


def kernel(outputs, targets, lengths, gamma):
    global _cached, last_results
    x = np.asarray(outputs)
    tgt = np.asarray(targets).astype(np.int64)
    lens = np.asarray(lengths).astype(np.int64)
    g = float(np.asarray(gamma))

    # Stage a copy with x[t, 0] <-> x[t, tgt[t]] swapped: the target
    # logit moves to class 0 (affine location), row exp-sums unchanged.
    x2 = np.array(x, dtype=np.float32, copy=True, order="C")
    ar = np.arange(T)
    v0 = x2[ar, 0].copy()
    x2[ar, 0] = x2[ar, tgt]
    x2[ar, tgt] = v0

    w = _host_weights(lens, g)

    # [p, col] -> local token index: t_loc = 512*(col//Q) + Q*p + (col%Q)
    cols = np.arange(NCOL, dtype=np.int64)
    psi = np.arange(P, dtype=np.int64)[:, None]
    t_loc = (P * Q) * (cols // Q) + Q * psi + (cols % Q)  # [P, NCOL]

    in_maps = []
    for c in range(NCORES):
        lo = c * TS
        w_l = w[lo:lo + TS]
        wt_c = np.ones((P, WCOL), dtype=np.float32)
        wt_c[:, :NCOL] = w_l[t_loc]
        wt_c[:, NCOL + 1] = 0.0  # Exp/Ln bias column
        in_maps.append({"x": x2[lo:lo + TS], "wt": wt_c})

    if _cached is None:
        nc = _build_bass_v2()
        _legalize_waits(nc)
        _strip_init_barrier(nc)
        _cached = nc
    nc = _cached

    def _run():
        return run_bass_kernel_spmd(nc, in_maps, core_ids=list(range(NCORES)))

    try:
        last_results = _run()
    except ModuleNotFoundError:
        # BASS_TRACE requested under axon but the image lacks
        # antenv.axon_hooks — rerun without tracing.
        _os.environ["BASS_NEVER_TRACE"] = "1"
        last_results = _run()
    except Exception:
        # transient device errors (e.g. NRT_EXEC_UNIT_UNRECOVERABLE) have
        # been observed on this fabric; retry once after a short pause
        import time as _time

        _time.sleep(5)
        last_results = _run()
    total = np.float64(0.0)
    for r in last_results.results:
        total += np.asarray(r["partial"], dtype=np.float64).sum()
    return np.float32(total / B)
